# revision 25
# baseline (speedup 1.0000x reference)
"""DistMult scoring kernel for Trainium2 (8 NeuronCores, Bass/Tile).

reference computation:
    rel = rel_embeds[rel_ids]                      # [B, D] gather
    scores = sum(head * rel * tail, axis=-1)       # [B]
    pos = min(scores[:n_pos], upper_bound)
    neg = max(scores[n_pos:], lower_bound)
    out = sigmoid(concat(pos, neg))

Strategy (sorted-chunk + selector-matmul; no device-side gather):
  * Host sorts rows by rel_id and packs them into chunks of CH=16 rows
    that all share one relation. Chunks are padded to a static layout:
    8 chunks per 128-row tile (chunk k -> partitions [16k, 16k+16)),
    T_PC=528 tiles per core. Pad slots carry zero h/t and +/-inf bounds,
    and are dropped on the host after the run.
  * Per tile the 8 chunk rel vectors [8, 256] are expanded to a full
    [128, 256] per-slot rel operand with a single TensorE matmul against
    a static one-hot selector lhsT [8, 128] (exact: one term per output).
    ScalarE casts the PSUM result to bf16 in SBUF.
  * DVE computes q = h*t and s = q*rel_bcast as big bf16 ops (2x mode)
    and reduces s over D with tensor_reduce into f32 scores.
  * Clamp with per-slot padded bounds (ub=+inf for neg rows, lb=-inf for
    pos rows), sigmoid, store. Host unpermutes.

Per-core slot r = p*T_PC + t (partition p, tile t) so every stream DMA
is contiguous per partition (16 tiles -> 8KB lines, 1MB per dma_start).
"""

import sys

for _p in ("/opt/trn_rl_repo",):
    if _p not in sys.path:
        sys.path.insert(0, _p)

import numpy as np

import concourse.bacc as bacc
import concourse.bass as bass
import concourse.mybir as mybir
import concourse.tile as tile
from concourse.bass_utils import run_bass_kernel_spmd

N_POS = 131072
N_NEG = 393216
B = N_POS + N_NEG  # 524288
D = 256
NUM_REL = 500
NCORES = 8
P = 128

CH = 32  # rows per chunk (uniform rel id within a chunk)
CPT = 4  # chunks per 128-row tile; chunk k -> partitions [32k, 32k+32)
T_PC = 528  # tiles per core
R = P * T_PC  # 67584 slots per core
GROUP = 16  # tiles per loop iteration
NG = T_PC // GROUP  # 33
CHUNKS_PER_CORE = T_PC * CPT  # 2112
TOTAL_CHUNKS = NCORES * CHUNKS_PER_CORE  # 16896 >= 500 + B/32 worst case




def build_program():
    bf = mybir.dt.bfloat16
    f32 = mybir.dt.float32
    mult = mybir.AluOpType.mult

    nc = bacc.Bacc(
        "TRN2", target_bir_lowering=False, debug=False, num_devices=NCORES
    )
    h = nc.declare_dram_parameter("h", [R, D], bf, isOutput=False)
    t_ = nc.declare_dram_parameter("t", [R, D], bf, isOutput=False)
    # rel[g, k, i*D:(i+1)*D] = rel vector of chunk k of tile g*GROUP+i
    rel = nc.declare_dram_parameter("rel", [NG, CPT, GROUP * D], bf, isOutput=False)
    # one-hot selector: sel[k, m] = 1 iff m//CH == k
    sel = nc.declare_dram_parameter("sel", [CPT, P], bf, isOutput=False)
    ub = nc.declare_dram_parameter("ub", [R], f32, isOutput=False)
    lb = nc.declare_dram_parameter("lb", [R], f32, isOutput=False)
    out = nc.declare_dram_parameter("out", [R], f32, isOutput=True)

    h_v = h[:].rearrange("(p t) d -> p t d", p=P)
    t_v = t_[:].rearrange("(p t) d -> p t d", p=P)
    rel_v = rel[:]
    ub_v = ub[:].rearrange("(p t) -> p t", p=P)
    lb_v = lb[:].rearrange("(p t) -> p t", p=P)
    out_v = out[:].rearrange("(p t) -> p t", p=P)

    with tile.TileContext(nc) as tc:
        with (
            tc.tile_pool(name="io", bufs=1) as io,
            tc.tile_pool(name="stream", bufs=3) as spool,
            tc.tile_pool(name="relp", bufs=2) as relp,
            tc.tile_pool(name="work", bufs=2) as work,
            tc.tile_pool(name="psum", bufs=2, space="PSUM") as psum,
        ):
            selt = io.tile([CPT, P], bf)
            nc.sync.dma_start(out=selt[:], in_=sel[:])
            scores = io.tile([P, T_PC], f32)
            ubt = io.tile([P, T_PC], f32)
            nc.sync.dma_start(out=ubt[:], in_=ub_v)
            lbt = io.tile([P, T_PC], f32)
            nc.sync.dma_start(out=lbt[:], in_=lb_v)

            def issue_dma(g):
                htile = spool.tile([P, GROUP * D], bf, tag="h", name=f"h{g}")
                nc.sync.dma_start(
                    out=htile[:].rearrange("p (i d) -> p i d", d=D),
                    in_=h_v[:, g * GROUP : (g + 1) * GROUP, :],
                )
                ttile = spool.tile([P, GROUP * D], bf, tag="t", name=f"t{g}")
                nc.sync.dma_start(
                    out=ttile[:].rearrange("p (i d) -> p i d", d=D),
                    in_=t_v[:, g * GROUP : (g + 1) * GROUP, :],
                )
                rtile = relp.tile([CPT, GROUP * D], bf, tag="r", name=f"r{g}")
                nc.sync.dma_start(out=rtile[:], in_=rel_v[g])
                return htile, ttile, rtile

            def issue_q(g, htile, ttile):
                qtile = work.tile(
                    [P, GROUP * D], bf, tag="q", bufs=3, name=f"q{g}"
                )
                nc.vector.tensor_tensor(
                    out=qtile[:], in0=htile[:], in1=ttile[:], op=mult
                )
                return qtile

            # steady clean all-DVE schedule (cross-engine offload measured
            # slower: in-order engines charge cross-engine waits as busy)
            ht, tt, rt = issue_dma(0)
            qtile = issue_q(0, ht, tt)
            pending = []
            for g in range(NG):
                if g + 1 < NG:
                    ht, tt, rt_next = issue_dma(g + 1)

                relb = work.tile([P, GROUP * D], bf, tag="b", name=f"b{g}")
                for half in range(2):
                    ps = psum.tile([P, 2048], f32, tag="ps", name=f"ps{g}_{half}")
                    for m4 in range(4):
                        i0 = half * 8 + m4 * 2
                        nc.tensor.matmul(
                            ps[:, m4 * 512 : (m4 + 1) * 512],
                            selt[:],
                            rt[:, i0 * D : (i0 + 2) * D],
                            start=True,
                            stop=True,
                        )
                    nc.scalar.activation(
                        out=relb[:, half * 2048 : (half + 1) * 2048],
                        in_=ps[:],
                        func=mybir.ActivationFunctionType.Copy,
                    )

                # s = q * relb (2x DVE TT per half), then shrink the 1x-only
                # reduce's input with 2x pairwise TT folds at full-group
                # granularity: 256 -> 64 per tile, reduce the last 64
                stile = work.tile([P, GROUP * D], bf, tag="s", name=f"s{g}")
                for half in range(2):
                    hs = slice(half * 8 * D, (half + 1) * 8 * D)
                    nc.vector.tensor_tensor(
                        out=stile[:, hs], in0=qtile[:, hs], in1=relb[:, hs],
                        op=mult,
                    )
                s3 = stile[:].rearrange("p (i d) -> p i d", d=D)
                f1 = work.tile(
                    [P, GROUP * 128], bf, tag="f1", bufs=3, name=f"f1_{g}"
                )
                f1v = f1[:].rearrange("p (i d) -> p i d", d=128)
                nc.vector.tensor_tensor(
                    out=f1v, in0=s3[:, :, 0:128], in1=s3[:, :, 128:256],
                    op=mybir.AluOpType.add,
                )
                # second fold on GpSimd (light: ~3us/group vs its ~5.8us/group
                # capacity); the dependent reduce runs a full group later on
                # DVE so neither engine head-of-line blocks on the other
                f2 = work.tile(
                    [P, GROUP * 64], bf, tag="f2", bufs=4, name=f"f2_{g}"
                )
                f2v = f2[:].rearrange("p (i d) -> p i d", d=64)
                nc.gpsimd.tensor_tensor(
                    out=f2v, in0=f1v[:, :, 0:64], in1=f1v[:, :, 64:128],
                    op=mybir.AluOpType.add,
                )

                if g + 1 < NG:
                    qtile = issue_q(g + 1, ht, tt)
                    rt = rt_next
                # reduce deferred TWO groups: the DVE->GpSimd->DVE dependency
                # has ~4.5us of sem round-trip latency; two groups (~14us of
                # DVE work) fully hides it
                pending.append((g, f2v))
                if len(pending) > 2:
                    rg, rf2v = pending.pop(0)
                    nc.vector.tensor_reduce(
                        out=scores[:, rg * GROUP : (rg + 1) * GROUP],
                        in_=rf2v,
                        axis=mybir.AxisListType.X,
                        op=mybir.AluOpType.add,
                    )
            for rg, rf2v in pending:
                nc.vector.tensor_reduce(
                    out=scores[:, rg * GROUP : (rg + 1) * GROUP],
                    in_=rf2v,
                    axis=mybir.AxisListType.X,
                    op=mybir.AluOpType.add,
                )

            c1 = io.tile([P, T_PC], f32)
            nc.vector.tensor_tensor(
                out=c1[:], in0=scores[:], in1=ubt[:], op=mybir.AluOpType.min
            )
            c2 = io.tile([P, T_PC], f32)
            nc.vector.tensor_tensor(
                out=c2[:], in0=c1[:], in1=lbt[:], op=mybir.AluOpType.max
            )
            sig = io.tile([P, T_PC], f32)
            nc.scalar.activation(
                out=sig[:], in_=c2[:], func=mybir.ActivationFunctionType.Sigmoid
            )
            nc.sync.dma_start(out=out_v, in_=sig[:])

    nc.compile()
    return nc


def make_in_maps(inputs: dict):
    """Sort rows by rel id, pack into uniform chunks, build per-core maps.

    Returns (in_maps, order, devrow): sorted row i (original row order[i])
    lands at global device slot devrow[i]; device output is read back with
    out[order] = res_all[devrow].
    """
    import ml_dtypes

    bf16 = ml_dtypes.bfloat16

    head = np.asarray(inputs["head_embeds"], dtype=np.float32).astype(bf16)
    tail = np.asarray(inputs["tail_embeds"], dtype=np.float32).astype(bf16)
    ids = np.asarray(inputs["rel_ids"]).astype(np.int64)
    lower = np.asarray(inputs["lower_bound"], dtype=np.float32)
    upper = np.asarray(inputs["upper_bound"], dtype=np.float32)
    table = np.asarray(inputs["rel_embeds"], dtype=np.float32).astype(bf16)

    order = np.argsort(ids, kind="stable")
    sids = ids[order]
    cnt = np.bincount(sids, minlength=NUM_REL)
    starts = np.zeros(NUM_REL, np.int64)
    starts[1:] = np.cumsum(cnt)[:-1]
    pos_in_rel = np.arange(B, dtype=np.int64) - starts[sids]
    chunks_per_rel = (cnt + CH - 1) // CH
    chunk_base = np.zeros(NUM_REL, np.int64)
    chunk_base[1:] = np.cumsum(chunks_per_rel)[:-1]
    n_chunks = int(chunks_per_rel.sum())
    assert n_chunks <= TOTAL_CHUNKS, n_chunks

    chunk_id = chunk_base[sids] + pos_in_rel // CH
    slot_in_chunk = pos_in_rel % CH

    core = chunk_id // CHUNKS_PER_CORE
    j = chunk_id % CHUNKS_PER_CORE
    t = j // CPT
    k = j % CPT
    p = k * CH + slot_in_chunk
    devrow = core * R + p * T_PC + t  # [B] global device slot per sorted row

    # rel id per chunk (uniform within a chunk; pad chunks use rel 0)
    rel_of_chunk = np.zeros(TOTAL_CHUNKS, np.int64)
    rel_of_chunk[chunk_id] = sids
    cc = np.arange(TOTAL_CHUNKS)
    core_c = cc // CHUNKS_PER_CORE
    j_c = cc % CHUNKS_PER_CORE
    t_c = j_c // CPT
    k_c = j_c % CPT
    relgrid = np.zeros((NCORES, NG, CPT, GROUP), np.int64)
    relgrid[core_c, t_c // GROUP, k_c, t_c % GROUP] = rel_of_chunk
    rel_dev = table[relgrid]  # [NCORES, NG, CPT, GROUP, D] bf16

    h_dev = np.zeros((NCORES * R, D), bf16)
    h_dev[devrow] = head[order]
    t_dev = np.zeros((NCORES * R, D), bf16)
    t_dev[devrow] = tail[order]

    ubv = np.full(B, np.inf, np.float32)
    lbv = np.full(B, -np.inf, np.float32)
    mask = order < N_POS
    ubv[mask] = upper[order[mask]]
    lbv[~mask] = lower[order[~mask] - N_POS]
    ub_dev = np.full(NCORES * R, np.inf, np.float32)
    lb_dev = np.full(NCORES * R, -np.inf, np.float32)
    ub_dev[devrow] = ubv
    lb_dev[devrow] = lbv

    sel = np.zeros((CPT, P), bf16)
    for kk in range(CPT):
        sel[kk, kk * CH : (kk + 1) * CH] = 1.0

    in_maps = []
    for c in range(NCORES):
        lo = c * R
        hi = lo + R
        in_maps.append(
            {
                "h": np.ascontiguousarray(h_dev[lo:hi]),
                "t": np.ascontiguousarray(t_dev[lo:hi]),
                "rel": np.ascontiguousarray(
                    rel_dev[c].reshape(NG, CPT, GROUP * D)
                ),
                "sel": sel,
                "ub": np.ascontiguousarray(ub_dev[lo:hi]),
                "lb": np.ascontiguousarray(lb_dev[lo:hi]),
            }
        )
    return in_maps, order, devrow


def _run(inputs: dict, trace: bool = False, tmpdir: str | None = None):
    nc = build_program()
    in_maps, order, devrow = make_in_maps(inputs)
    res = run_bass_kernel_spmd(
        nc, in_maps, list(range(NCORES)), trace=trace, tmpdir=tmpdir
    )
    res_all = np.concatenate(
        [np.asarray(res.results[c]["out"]) for c in range(NCORES)]
    )
    out = np.empty(B, np.float32)
    out[order] = res_all[devrow]
    return out, res


def kernel(**inputs) -> np.ndarray:
    out, _ = _run(inputs)
    return out


# revision 27
# speedup vs baseline: 1.1884x; 1.1884x over previous
"""DistMult scoring kernel for Trainium2 (8 NeuronCores, Bass/Tile).

reference computation:
    rel = rel_embeds[rel_ids]                      # [B, D] gather
    scores = sum(head * rel * tail, axis=-1)       # [B]
    pos = min(scores[:n_pos], upper_bound)
    neg = max(scores[n_pos:], lower_bound)
    out = sigmoid(concat(pos, neg))

Strategy (sorted-chunk + selector-matmul; no device-side gather):
  * Host sorts rows by rel_id and packs them into chunks of CH=16 rows
    that all share one relation. Chunks are padded to a static layout:
    8 chunks per 128-row tile (chunk k -> partitions [16k, 16k+16)),
    T_PC=528 tiles per core. Pad slots carry zero h/t and +/-inf bounds,
    and are dropped on the host after the run.
  * Per tile the 8 chunk rel vectors [8, 256] are expanded to a full
    [128, 256] per-slot rel operand with a single TensorE matmul against
    a static one-hot selector lhsT [8, 128] (exact: one term per output).
    ScalarE casts the PSUM result to bf16 in SBUF.
  * DVE computes q = h*t and s = q*rel_bcast as big bf16 ops (2x mode)
    and reduces s over D with tensor_reduce into f32 scores.
  * Clamp with per-slot padded bounds (ub=+inf for neg rows, lb=-inf for
    pos rows), sigmoid, store. Host unpermutes.

Per-core slot r = p*T_PC + t (partition p, tile t) so every stream DMA
is contiguous per partition (16 tiles -> 8KB lines, 1MB per dma_start).
"""

import sys

for _p in ("/opt/trn_rl_repo",):
    if _p not in sys.path:
        sys.path.insert(0, _p)

import numpy as np

import concourse.bacc as bacc
import concourse.bass as bass
import concourse.mybir as mybir
import concourse.tile as tile
from concourse.bass_utils import run_bass_kernel_spmd

N_POS = 131072
N_NEG = 393216
B = N_POS + N_NEG  # 524288
D = 256
NUM_REL = 500
NCORES = 8
P = 128

CH = 32  # rows per chunk (uniform rel id within a chunk)
CPT = 4  # chunks per 128-row tile; chunk k -> partitions [32k, 32k+32)
T_PC = 528  # tiles per core
R = P * T_PC  # 67584 slots per core
GROUP = 16  # tiles per loop iteration
NG = T_PC // GROUP  # 33
CHUNKS_PER_CORE = T_PC * CPT  # 2112
TOTAL_CHUNKS = NCORES * CHUNKS_PER_CORE  # 16896 >= 500 + B/32 worst case




def build_program():
    bf = mybir.dt.bfloat16
    f32 = mybir.dt.float32
    mult = mybir.AluOpType.mult

    nc = bacc.Bacc(
        "TRN2", target_bir_lowering=False, debug=False, num_devices=NCORES
    )
    h = nc.declare_dram_parameter("h", [R, D], bf, isOutput=False)
    t_ = nc.declare_dram_parameter("t", [R, D], bf, isOutput=False)
    # rel[g, k, i*D:(i+1)*D] = rel vector of chunk k of tile g*GROUP+i
    rel = nc.declare_dram_parameter("rel", [NG, CPT, GROUP * D], bf, isOutput=False)
    # one-hot selector: sel[k, m] = 1 iff m//CH == k
    sel = nc.declare_dram_parameter("sel", [CPT, P], bf, isOutput=False)
    ub = nc.declare_dram_parameter("ub", [R], f32, isOutput=False)
    lb = nc.declare_dram_parameter("lb", [R], f32, isOutput=False)
    out = nc.declare_dram_parameter("out", [R], f32, isOutput=True)

    h_v = h[:].rearrange("(p t) d -> p t d", p=P)
    t_v = t_[:].rearrange("(p t) d -> p t d", p=P)
    rel_v = rel[:]
    ub_v = ub[:].rearrange("(p t) -> p t", p=P)
    lb_v = lb[:].rearrange("(p t) -> p t", p=P)
    out_v = out[:].rearrange("(p t) -> p t", p=P)

    with tile.TileContext(nc) as tc:
        with (
            tc.tile_pool(name="io", bufs=1) as io,
            tc.tile_pool(name="stream", bufs=3) as spool,
            tc.tile_pool(name="relp", bufs=2) as relp,
            tc.tile_pool(name="work", bufs=2) as work,
            tc.tile_pool(name="psum", bufs=2, space="PSUM") as psum,
        ):
            selt = io.tile([CPT, P], bf)
            nc.sync.dma_start(out=selt[:], in_=sel[:])
            scores = io.tile([P, T_PC], f32)
            ubt = io.tile([P, T_PC], f32)
            nc.sync.dma_start(out=ubt[:], in_=ub_v)
            lbt = io.tile([P, T_PC], f32)
            nc.sync.dma_start(out=lbt[:], in_=lb_v)

            def issue_dma(g):
                htile = spool.tile([P, GROUP * D], bf, tag="h", name=f"h{g}")
                nc.sync.dma_start(
                    out=htile[:].rearrange("p (i d) -> p i d", d=D),
                    in_=h_v[:, g * GROUP : (g + 1) * GROUP, :],
                )
                ttile = spool.tile([P, GROUP * D], bf, tag="t", name=f"t{g}")
                nc.sync.dma_start(
                    out=ttile[:].rearrange("p (i d) -> p i d", d=D),
                    in_=t_v[:, g * GROUP : (g + 1) * GROUP, :],
                )
                rtile = relp.tile([CPT, GROUP * D], bf, tag="r", name=f"r{g}")
                nc.sync.dma_start(out=rtile[:], in_=rel_v[g])
                return htile, ttile, rtile

            def issue_q(g, htile, ttile):
                qtile = work.tile(
                    [P, GROUP * D], bf, tag="q", bufs=3, name=f"q{g}"
                )
                nc.vector.tensor_tensor(
                    out=qtile[:], in0=htile[:], in1=ttile[:], op=mult
                )
                return qtile

            # steady clean all-DVE schedule (cross-engine offload measured
            # slower: in-order engines charge cross-engine waits as busy)
            ht, tt, rt = issue_dma(0)
            qtile = issue_q(0, ht, tt)
            for g in range(NG):
                if g + 1 < NG:
                    ht, tt, rt_next = issue_dma(g + 1)

                relb = work.tile([P, GROUP * D], bf, tag="b", name=f"b{g}")
                for half in range(2):
                    ps = psum.tile([P, 2048], f32, tag="ps", name=f"ps{g}_{half}")
                    for m4 in range(4):
                        i0 = half * 8 + m4 * 2
                        nc.tensor.matmul(
                            ps[:, m4 * 512 : (m4 + 1) * 512],
                            selt[:],
                            rt[:, i0 * D : (i0 + 2) * D],
                            start=True,
                            stop=True,
                        )
                    nc.scalar.activation(
                        out=relb[:, half * 2048 : (half + 1) * 2048],
                        in_=ps[:],
                        func=mybir.ActivationFunctionType.Copy,
                    )

                # s = q * relb (2x DVE TT per half), then shrink the 1x-only
                # reduce's input with 2x pairwise TT folds at full-group
                # granularity: 256 -> 64 per tile, reduce the last 64
                stile = work.tile([P, GROUP * D], bf, tag="s", name=f"s{g}")
                for half in range(2):
                    hs = slice(half * 8 * D, (half + 1) * 8 * D)
                    nc.vector.tensor_tensor(
                        out=stile[:, hs], in0=qtile[:, hs], in1=relb[:, hs],
                        op=mult,
                    )
                s3 = stile[:].rearrange("p (i d) -> p i d", d=D)
                f1 = work.tile([P, GROUP * 128], bf, tag="f1", name=f"f1_{g}")
                f1v = f1[:].rearrange("p (i d) -> p i d", d=128)
                nc.vector.tensor_tensor(
                    out=f1v, in0=s3[:, :, 0:128], in1=s3[:, :, 128:256],
                    op=mybir.AluOpType.add,
                )
                f2 = work.tile([P, GROUP * 64], bf, tag="f2", name=f"f2_{g}")
                f2v = f2[:].rearrange("p (i d) -> p i d", d=64)
                nc.vector.tensor_tensor(
                    out=f2v, in0=f1v[:, :, 0:64], in1=f1v[:, :, 64:128],
                    op=mybir.AluOpType.add,
                )
                nc.vector.tensor_reduce(
                    out=scores[:, g * GROUP : (g + 1) * GROUP],
                    in_=f2v,
                    axis=mybir.AxisListType.X,
                    op=mybir.AluOpType.add,
                )

                if g + 1 < NG:
                    qtile = issue_q(g + 1, ht, tt)
                    rt = rt_next

            c1 = io.tile([P, T_PC], f32)
            nc.vector.tensor_tensor(
                out=c1[:], in0=scores[:], in1=ubt[:], op=mybir.AluOpType.min
            )
            c2 = io.tile([P, T_PC], f32)
            nc.vector.tensor_tensor(
                out=c2[:], in0=c1[:], in1=lbt[:], op=mybir.AluOpType.max
            )
            sig = io.tile([P, T_PC], f32)
            nc.scalar.activation(
                out=sig[:], in_=c2[:], func=mybir.ActivationFunctionType.Sigmoid
            )
            nc.sync.dma_start(out=out_v, in_=sig[:])

    nc.compile()
    return nc


def make_in_maps(inputs: dict):
    """Sort rows by rel id, pack into uniform chunks, build per-core maps.

    Returns (in_maps, order, devrow): sorted row i (original row order[i])
    lands at global device slot devrow[i]; device output is read back with
    out[order] = res_all[devrow].
    """
    import ml_dtypes

    bf16 = ml_dtypes.bfloat16

    head = np.asarray(inputs["head_embeds"], dtype=np.float32).astype(bf16)
    tail = np.asarray(inputs["tail_embeds"], dtype=np.float32).astype(bf16)
    ids = np.asarray(inputs["rel_ids"]).astype(np.int64)
    lower = np.asarray(inputs["lower_bound"], dtype=np.float32)
    upper = np.asarray(inputs["upper_bound"], dtype=np.float32)
    table = np.asarray(inputs["rel_embeds"], dtype=np.float32).astype(bf16)

    order = np.argsort(ids, kind="stable")
    sids = ids[order]
    cnt = np.bincount(sids, minlength=NUM_REL)
    starts = np.zeros(NUM_REL, np.int64)
    starts[1:] = np.cumsum(cnt)[:-1]
    pos_in_rel = np.arange(B, dtype=np.int64) - starts[sids]
    chunks_per_rel = (cnt + CH - 1) // CH
    chunk_base = np.zeros(NUM_REL, np.int64)
    chunk_base[1:] = np.cumsum(chunks_per_rel)[:-1]
    n_chunks = int(chunks_per_rel.sum())
    assert n_chunks <= TOTAL_CHUNKS, n_chunks

    chunk_id = chunk_base[sids] + pos_in_rel // CH
    slot_in_chunk = pos_in_rel % CH

    core = chunk_id // CHUNKS_PER_CORE
    j = chunk_id % CHUNKS_PER_CORE
    t = j // CPT
    k = j % CPT
    p = k * CH + slot_in_chunk
    devrow = core * R + p * T_PC + t  # [B] global device slot per sorted row

    # rel id per chunk (uniform within a chunk; pad chunks use rel 0)
    rel_of_chunk = np.zeros(TOTAL_CHUNKS, np.int64)
    rel_of_chunk[chunk_id] = sids
    cc = np.arange(TOTAL_CHUNKS)
    core_c = cc // CHUNKS_PER_CORE
    j_c = cc % CHUNKS_PER_CORE
    t_c = j_c // CPT
    k_c = j_c % CPT
    relgrid = np.zeros((NCORES, NG, CPT, GROUP), np.int64)
    relgrid[core_c, t_c // GROUP, k_c, t_c % GROUP] = rel_of_chunk
    rel_dev = table[relgrid]  # [NCORES, NG, CPT, GROUP, D] bf16

    h_dev = np.zeros((NCORES * R, D), bf16)
    h_dev[devrow] = head[order]
    t_dev = np.zeros((NCORES * R, D), bf16)
    t_dev[devrow] = tail[order]

    ubv = np.full(B, np.inf, np.float32)
    lbv = np.full(B, -np.inf, np.float32)
    mask = order < N_POS
    ubv[mask] = upper[order[mask]]
    lbv[~mask] = lower[order[~mask] - N_POS]
    ub_dev = np.full(NCORES * R, np.inf, np.float32)
    lb_dev = np.full(NCORES * R, -np.inf, np.float32)
    ub_dev[devrow] = ubv
    lb_dev[devrow] = lbv

    sel = np.zeros((CPT, P), bf16)
    for kk in range(CPT):
        sel[kk, kk * CH : (kk + 1) * CH] = 1.0

    in_maps = []
    for c in range(NCORES):
        lo = c * R
        hi = lo + R
        in_maps.append(
            {
                "h": np.ascontiguousarray(h_dev[lo:hi]),
                "t": np.ascontiguousarray(t_dev[lo:hi]),
                "rel": np.ascontiguousarray(
                    rel_dev[c].reshape(NG, CPT, GROUP * D)
                ),
                "sel": sel,
                "ub": np.ascontiguousarray(ub_dev[lo:hi]),
                "lb": np.ascontiguousarray(lb_dev[lo:hi]),
            }
        )
    return in_maps, order, devrow


def _run(inputs: dict, trace: bool = False, tmpdir: str | None = None):
    nc = build_program()
    in_maps, order, devrow = make_in_maps(inputs)
    res = run_bass_kernel_spmd(
        nc, in_maps, list(range(NCORES)), trace=trace, tmpdir=tmpdir
    )
    res_all = np.concatenate(
        [np.asarray(res.results[c]["out"]) for c in range(NCORES)]
    )
    out = np.empty(B, np.float32)
    out[order] = res_all[devrow]
    return out, res


def kernel(**inputs) -> np.ndarray:
    out, _ = _run(inputs)
    return out
